# revision 41
# baseline (speedup 1.0000x reference)
"""Trainium2 Bass kernel for the dual-softmax cross-attention module.

Sharding: 8 cores = batch (4) x head-half (2).  Core c handles batch c//2 and
heads 4*(c%2) .. 4*(c%2)+4.  Each core computes Q/K/V projections for its
head-group, the 2048x2048 score matrix per head, one shared E = exp(s/8)
(both softmaxes are shift-invariant; scores are O(1) so no max subtraction),
contexts for both streams, exchanges context halves with its pair core via a
2-core AllGather, and produces a disjoint 256-channel slice of both outputs.

fp8 (e4m3) edition: all matmul operands are fp8 with fp32 PSUM accumulation.
Chained-contraction matmuls (QKV/O projections, ctx1, ctx2) use DoubleRow
perf mode (two 128-row K blocks per instruction).  E = exp(s/8) is written
as fp8 and transposed for ctx1 via 2-byte DMA transposes of BYTE PAIRS; a
k-side interleave permutation sigma (kslot = 256a+2p+ko <-> token
256a+128ko+p) makes the transposed byte pairs land exactly on v1's natural
token blocks, so ctx1's DoubleRow operands line up with no data shuffles.
The k-side permutation is compensated host-side on x2's residual and o2.
Weights are scaled x32 for e4m3 range; projections un-scale via ACT.
"""

import sys

for _p in ("/opt/trn_rl_repo", "/opt/pypackages"):
    if _p not in sys.path:
        sys.path.insert(0, _p)

import numpy as np
import ml_dtypes

import concourse.bass as bass
import concourse.tile as tile
from concourse import bacc, mybir
from concourse.bass_utils import run_bass_kernel_spmd

F32 = mybir.dt.float32
BF16 = mybir.dt.bfloat16
F8 = mybir.dt.float8e4
AF = mybir.ActivationFunctionType
AX = mybir.AxisListType
DR = mybir.MatmulPerfMode.DoubleRow

N_CORES = 8
B = 4          # batch
C = 512        # channels
N = 2048       # tokens (8*16*16)
H = 8          # heads
DH = 64        # head dim
HL = 4         # heads per core
CL = 256       # channels per core (head-group)
NT = N // 128  # 16 token tiles
CT = C // 128  # 4 channel tiles
WSC = 32.0     # fp8 weight pre-scale (QKV projections)
ISC = 1.0 / WSC
OSC = 4.0      # fp8 Wo pre-scale; gathered ctx is stored as ctx/OSC so the
OISC = 1.0 / OSC  # output projection psum needs no un-scaling

_F8 = ml_dtypes.float8_e4m3
_BF = ml_dtypes.bfloat16

# k-side interleave: kslot -> token
_ks = np.arange(N)
SIGMA = ((_ks >> 8) << 8) + ((_ks & 1) << 7) + ((_ks & 255) >> 1)


def _build():
    nc = bacc.Bacc("TRN2", target_bir_lowering=False, debug=False,
                   num_devices=N_CORES)

    def din(name, shape, dt=F8):
        return nc.dram_tensor(name, shape, dt, kind="ExternalInput").ap()

    x1b = din("x1b", [CT, 128, N])          # x1[b] channel-major, fp8
    x2b = din("x2b", [CT, 128, N])
    wq = din("wq", [128, CT, CL])           # column slice of Wq*32, pre-permuted
    wk = din("wk", [128, CT, CL])
    wv1 = din("wv1", [128, CT, CL])
    wv2 = din("wv2", [128, CT, CL])
    wo1 = din("wo1", [128, CT, CL])         # Wo*32 columns, CT blocks in [0,2,1,3]
    wo2 = din("wo2", [128, CT, CL])
    bq = din("bq", [128, 2, 1], F32)        # bias slices per M-tile (true scale)
    bk = din("bk", [128, 2, 1], F32)
    bv1 = din("bv1", [1, CL])               # 32*bv, fp8
    bv2 = din("bv2", [1, CL])
    x1r = din("x1r", [2, 128, N], F32)      # x1[b] residual slice + bo1
    x2r = din("x2r", [2, 128, N], F32)      # sigma-permuted + bo2

    o1 = nc.dram_tensor("o1", [2, 128, N], BF16, kind="ExternalOutput").ap()
    o2 = nc.dram_tensor("o2", [2, 128, N], BF16, kind="ExternalOutput").ap()

    with tile.TileContext(nc) as tc:
        _emit(nc, tc, locals())
    nc.compile()
    return nc


def _emit(nc, tc, t):
    x1b, x2b = t["x1b"], t["x2b"]
    wq, wk, wv1, wv2 = t["wq"], t["wk"], t["wv1"], t["wv2"]
    wo1, wo2 = t["wo1"], t["wo2"]
    bq, bk, bv1, bv2 = t["bq"], t["bk"], t["bv1"], t["bv2"]
    x1r, x2r, o1, o2 = t["x1r"], t["x2r"], t["o1"], t["o2"]

    from contextlib import ExitStack
    ctx = ExitStack()
    with ctx:
        persist = ctx.enter_context(tc.tile_pool(name="persist", bufs=1))
        small = ctx.enter_context(tc.tile_pool(name="small", bufs=8))
        vp_pool = ctx.enter_context(tc.tile_pool(name="vp", bufs=4))
        dram = ctx.enter_context(tc.tile_pool(name="dram", bufs=2, space="DRAM"))

        # ---- persistent SBUF tensors ----
        w_all = persist.tile([128, 6, CT, CL], F8, tag="wall")
        wq_s, wk_s, wv1_s, wv2_s, wo1_s, wo2_s = (w_all[:, i, :, :]
                                                  for i in range(6))
        bqk_s = persist.tile([128, 4, 1], F32, tag="bqk")
        bq_s, bk_s = bqk_s[:, 0:2, :], bqk_s[:, 2:4, :]
        ones_full = persist.tile([128, N], F8, tag="ones", name="ones_full")
        ones_s = ones_full[0:1, :]
        vb_s = persist.tile([1, 2, CL], F8, tag="vb")
        bv1_s, bv2_s = vb_s[:, 0, :], vb_s[:, 1, :]
        miscb = persist.tile([128, 8], BF16, tag="miscb")
        onec_s = miscb[:, 0:1]
        qt_s = persist.tile([128, 2, N], F8, tag="qt")    # Q^T  (chan-major)
        kt_s = persist.tile([128, 2, N], F8, tag="kt")    # K^T  (sigma k-order)
        v1tok = persist.tile([128, NT, CL], F8, tag="v1tok")  # token-major V1
        v2tok = persist.tile([128, NT, CL], F8, tag="v2tok")
        cm = {}  # gathered ctx^T tiles; pool opened once xb tiles retire

        # Q/K weights first (they gate the first projections); V/O later
        nc.sync.dma_start(w_all[:, 0, :, :], wq[:, :, :])
        nc.scalar.dma_start(w_all[:, 1, :, :], wk[:, :, :])
        nc.scalar.dma_start(bq_s[:, :, :], bq[:, :, :])
        nc.scalar.dma_start(bk_s[:, :, :], bk[:, :, :])
        nc.vector.memset(ones_s[:, :], 1.0)
        nc.vector.memset(onec_s[:, :], 1.0)

        # ---- P1: x loads + Q/K projections (V projections are interleaved
        # into head 0's qtile loop, using the then-idle ctx1 psum slot) ----
        p2 = ExitStack()
        eslab = p2.enter_context(tc.tile_pool(name="eslab", bufs=8))
        et_pool = p2.enter_context(tc.tile_pool(name="et", bufs=1))
        gsrc_pool = p2.enter_context(tc.tile_pool(name="gsrc", bufs=4))
        csrow_pool = p2.enter_context(tc.tile_pool(name="csrow", bufs=1))
        p1 = ExitStack()
        pj_ps = p1.enter_context(tc.tile_pool(name="pj_ps", bufs=2, space="PSUM"))
        xb_stack = ExitStack()
        xb_pool = xb_stack.enter_context(tc.tile_pool(name="xb", bufs=2))
        xts = {}
        for xi, xb_dram in enumerate((x1b, x2b)):
            xts[xi] = xb_pool.tile([128, CT, N], F8, tag="xb", name=f"xt{xi}")
            for ti in range(CT):
                eng = nc.sync if (ti + xi) % 2 == 0 else nc.scalar
                eng.dma_start(xts[xi][:, ti, :], xb_dram[ti, :, :])
        # V/O weights + biases land behind the x tiles on the rings
        for i, src in ((2, wv1), (3, wv2), (4, wo1), (5, wo2)):
            eng = nc.sync if i % 2 == 0 else nc.scalar
            eng.dma_start(w_all[:, i, :, :], src[:, :, :])
        nc.scalar.dma_start(bv1_s[:, :], bv1[:, :])
        nc.scalar.dma_start(bv2_s[:, :], bv2[:, :])
        # chan-major Q/K:  out[cl, n] = (1/32) sum_cin 32W[cin, cl] x[cin, n]
        for xi, w_qk, b_qk, qk_dst, perm in ((0, wq_s, bq_s, qt_s, False),
                                             (1, wk_s, bk_s, kt_s, True)):
            for m in range(2):
                for half in range(2):
                    ps = pj_ps.tile([128, 1024], F32, tag="pj")
                    for ch in range(2):
                        off = half * 1024 + ch * 512
                        for tp in range(2):
                            nc.tensor.matmul(
                                ps[:, ch * 512:(ch + 1) * 512],
                                w_qk[:, 2 * tp:2 * tp + 2, m * 128:(m + 1) * 128],
                                xts[xi][:, 2 * tp:2 * tp + 2, off:off + 512],
                                start=(tp == 0), stop=(tp == 1), perf_mode=DR)
                    dst = qk_dst[:, m, half * 1024:(half + 1) * 1024]
                    src = ps[:, :]
                    if perm:
                        # sigma interleave: token 256A+128ko+p -> col 256A+2p+ko
                        dst = dst.rearrange("c (A p ko) -> c A ko p",
                                            A=4, p=128, ko=2)
                        src = src.rearrange("c (A ko p) -> c A ko p",
                                            A=4, ko=2, p=128)
                    nc.vector.tensor_scalar(
                        dst, src, ISC, b_qk[:, m, :],
                        mybir.AluOpType.mult, mybir.AluOpType.add)
        p1.close()

        def emit_v_proj(xi, w_v, b_v, v_dst, nt, vps_pool):
            # token-major V:  out[n, cl] = (1/32)(sum_cin x 32W + 32bv)
            ps = vps_pool.tile([128, 512], F32, tag="c1", name=f"vps{xi}_{nt}")
            for tp in range(2):
                nc.tensor.matmul(
                    ps[:, 0:CL],
                    xts[xi][:, 2 * tp:2 * tp + 2, nt * 128:(nt + 1) * 128],
                    w_v[:, 2 * tp:2 * tp + 2, :],
                    start=(tp == 0), stop=False, perf_mode=DR)
            nc.tensor.matmul(ps[:, 0:CL], ones_s[:, nt * 128:(nt + 1) * 128],
                             b_v[:, :], start=False, stop=True)
            nc.vector.tensor_scalar_mul(v_dst[:, nt, :], ps[:, 0:CL], ISC)

        # ---- P2: per-head attention, software-pipelined across heads ----
        sc_ps = p2.enter_context(tc.tile_pool(name="sc_ps", bufs=2, space="PSUM"))
        c2_ps = p2.enter_context(tc.tile_pool(name="c2_ps", bufs=1, space="PSUM"))
        c1_ps = p2.enter_context(tc.tile_pool(name="c1_ps", bufs=2, space="PSUM"))

        st = {}  # per-head pipeline state

        def head_slices(hl):
            g, poff = hl // 2, 64 * (hl % 2)
            return (qt_s[poff:poff + 64, g, :], kt_s[poff:poff + 64, g, :], poff)

        def emit_scores_exp(hl, qt):
            q_l, k_l, _ = head_slices(hl)
            s = st[hl]
            if qt % 2 == 0:
                s["esp"][qt // 2] = eslab.tile([128, 2, N], F8, tag="es",
                                               name=f"es{hl}_{qt // 2}")
            es = s["esp"][qt // 2][:, qt % 2, :]
            sq = small.tile([128, 24], F32, tag="sq", bufs=4,
                            name=f"sq{hl}_{qt}")
            rs_p, rs, rr = sq[:, 0:3], sq[:, 4:5], sq[:, 5:6]
            for u in range(4):
                ps = sc_ps.tile([128, 512], F32, tag="sc", name=f"sps{u}")
                nc.tensor.matmul(ps[:, :], q_l[:, qt * 128:(qt + 1) * 128],
                                 k_l[:, u * 512:(u + 1) * 512],
                                 start=True, stop=True)
                # rowsum split: chunk 0 rides the ACT fused accumulator,
                # chunks 1-3 are reduced on DVE in one op below
                nc.scalar.activation(es[:, u * 512:(u + 1) * 512], ps[:, :],
                                     AF.Exp, scale=0.125,
                                     accum_out=(rs_p[:, 0:1]
                                                if u == 0 else None))
            nc.vector.reduce_sum(out=rs_p[:, 1:2], in_=es[:, 512:2048],
                                 axis=AX.X)
            nc.vector.tensor_add(rs[:, :], rs_p[:, 0:1], rs_p[:, 1:2])
            nc.vector.reciprocal(rr[:, :], rs[:, :])
            if qt % 2 == 0:
                s["v2pk"] = vp_pool.tile([128, 2, 80], F8, tag="v2p",
                                         bufs=2, name=f"v2pk{hl}_{qt}")
            v2p = s["v2pk"][:, qt % 2, :]
            nc.vector.tensor_scalar_mul(
                v2p[0:128, 0:DH], v2tok[:, qt, hl * DH:(hl + 1) * DH], rr[:, :])
            nc.vector.memset(v2p[0:128, DH:DH + 2], 1.0)
            if qt % 2 == 1:
                s["v2pairs"][qt // 2] = s["v2pk"]

        def emit_ctx2(hl, pj):
            # one qt-pair of ctx2 via DoubleRow (also accumulates colsum row 64)
            s = st[hl]
            esp = s["esp"][pj]
            v2pk = s["v2pairs"][pj]
            for ch in range(4):
                nc.tensor.matmul(
                    s["cps2"][0:DH + 2, ch * 512:(ch + 1) * 512],
                    v2pk[:, :, 0:DH + 2],
                    esp[:, :, ch * 512:(ch + 1) * 512],
                    start=(pj == 0), stop=(pj == NT // 2 - 1), perf_mode=DR)

        def emit_transpose(hl, qt):
            # byte-pair transpose: es fp8 [128q, 2048k] viewed as bf16
            # [128, 1024] -> et[:, qt] bf16 [128, 8, 128]; et fp8 view holds
            # E^T with (token-block 2A+ko, p) at fp8 byte (p, A, 2b+ko)
            s = st[hl]
            if s["et"] is None:
                s["et"] = et_pool.tile([128, NT, 8, 128], BF16, tag="et",
                                       name=f"et{hl}")
            nc.sync.dma_start(
                s["et"][:, qt, :, :],
                s["esp"][qt // 2][:, qt % 2, :].bitcast(BF16),
                transpose=True)

        def emit_epilogue_a(hl):
            # copy colsum row out of psum FIRST (it gates the next head's
            # psum reuse), then evac ctx2
            s = st[hl]
            csrow = csrow_pool.tile([65, N], BF16, tag="csr", name=f"csr{hl}")
            s["csrow"] = csrow
            nc.vector.tensor_copy(csrow[64:65, :], s["cps2"][64:65, :])
            gs2 = gsrc_pool.tile([64, N], F8, tag="gs", name=f"gs2_{hl}")
            s["gs2"] = gs2
            nc.vector.tensor_scalar_mul(gs2[:, :], s["cps2"][0:64, :], OISC)
            s["gs1"] = gsrc_pool.tile([64, N], F8, tag="gs",
                                      name=f"gs1_{hl}")

        def emit_epilogue_b(hl):
            # colsum row -> column via 16 K=1 matmuls with sigma-strided
            # lhsT (col nt of cs_ps = colsum of natural token block nt)
            s = st[hl]
            cs_ps = sc_ps.tile([128, 512], F32, tag="sc", name=f"cs_ps{hl}")
            csr = s["csrow"][64:65, :].rearrange("r (A p ko) -> r A ko p",
                                                 A=8, p=128, ko=2)
            for nt in range(NT):
                nc.tensor.matmul(cs_ps[:, nt:nt + 1],
                                 csr[:, nt // 2, nt % 2, :],
                                 onec_s[64:65, :], start=True, stop=True)
            cr_t = small.tile([128, NT], F32, tag="cr", bufs=2, name=f"cr{hl}")
            nc.vector.reciprocal(cr_t[:, :], cs_ps[:, 0:NT])
            v1pk = vp_pool.tile([128, NT, DH], F8, tag="v1p", bufs=2,
                                name=f"v1pk{hl}")
            s["v1pk"] = v1pk
            for nt in range(NT):
                nc.vector.tensor_scalar_mul(
                    v1pk[:, nt, :], v1tok[:, nt, hl * DH:(hl + 1) * DH],
                    cr_t[:, nt:nt + 1])

        def emit_ctx1_step(hl, step):
            # step 0..11: ch = step//3, A-pair third = step%3 (3/3/2 pairs)
            s = st[hl]
            ch, third = step // 3, step % 3
            a_lo, a_hi = ((0, 3), (3, 6), (6, 8))[third]
            if third == 0:
                s["c1"][ch] = c1_ps.tile([64, 512], F32, tag="c1",
                                         name=f"c1_{hl}_{ch}")
            et8 = s["et"][:, 4 * ch:4 * (ch + 1), :, :].bitcast(F8)
            for a in range(a_lo, a_hi):
                nc.tensor.matmul(
                    s["c1"][ch][:, :],
                    s["v1pk"][:, 2 * a:2 * a + 2, :],
                    et8[:, :, a, :].rearrange("c q (b ko) -> c ko q b",
                                              b=128, ko=2),
                    start=(a == 0), stop=(a == 7), perf_mode=DR)
            if third == 2:
                nc.vector.tensor_scalar_mul(
                    s["gs1"][:, ch * 512:(ch + 1) * 512],
                    s["c1"][ch][:, :], OISC)

        def emit_gather(hls, half=None):
            # hls: heads whose ctx ships in one collective.  half (single
            # head only): 0 = ctx2 rows, 1 = ctx1 rows.  All SBUF<->DRAM
            # legs ride the gpsimd SWDGE ring so the collective's completion
            # wait never head-of-line-blocks the HWDGE rings.
            nh = len(hls)
            nr = 128 * nh if half is None else 64
            sfx = f"{'_'.join(map(str, hls))}_{half}"
            gin = dram.tile([nr, N], F8, tag="gin", name=f"gin{sfx}")
            gout = dram.tile([2, nr, N], F8, tag="gout", bufs=4,
                             name=f"gout{sfx}")
            for i, hl in enumerate(hls):
                s = st[hl]
                if half in (None, 0):
                    nc.gpsimd.dma_start(gin[i * 128:i * 128 + 64, :]
                                        if half is None else gin[0:64, :],
                                        s["gs2"][:, :])
                if half in (None, 1):
                    ro = i * 128 + 64 if half is None else 0
                    nc.gpsimd.dma_start(gin[ro:ro + 64, :], s["gs1"][:, :])
            nc.gpsimd.collective_compute(
                "AllGather", mybir.AluOpType.bypass,
                replica_groups=[[0, 1], [2, 3], [4, 5], [6, 7]],
                ins=[gin.opt()], outs=[gout.opt()])
            for r in range(2):
                for i, hl in enumerate(hls):
                    _, _, poff = head_slices(hl)
                    # cm channel-block order [0,2,1,3] (host compensates in
                    # Wo): blocks {0,1} = heads 0-1 -> early DR pair
                    tt = 2 * (hl // 2) + r
                    if half in (None, 0):
                        ro = i * 128 if half is None else 0
                        nc.gpsimd.dma_start(cm["2"][poff:poff + 64, tt, :],
                                            gout[r, ro:ro + 64, :])
                    if half in (None, 1):
                        ro = i * 128 + 64 if half is None else 0
                        nc.gpsimd.dma_start(cm["1"][poff:poff + 64, tt, :],
                                            gout[r, ro:ro + 64, :])

        def emit_head_qt(hl, qt):
            # one qtile of head hl + interleaved epilogue work of head hl-1
            # (or, for head 0, the V projections)
            if hl == 0:
                emit_v_proj(1, wv2_s, bv2_s, v2tok, qt, c1_ps)
            emit_scores_exp(hl, qt)
            if hl == 0:
                emit_v_proj(0, wv1_s, bv1_s, v1tok, qt, c1_ps)
            else:
                if qt == 1:
                    emit_epilogue_b(hl - 1)
                elif 2 <= qt <= 13:
                    emit_ctx1_step(hl - 1, qt - 2)
                elif qt == 14 and hl >= 2:
                    # heads 0+1 ship together once head 1's ctx1 is done
                    emit_gather((0, 1) if hl == 2 else (hl - 1,))
            if qt >= 2 and qt % 2 == 0:
                emit_ctx2(hl, qt // 2 - 1)
            if qt >= 2:
                emit_transpose(hl, qt - 2)

        for hl in range(HL):
            st[hl] = {"esp": {}, "v2pairs": {}, "c1": {}, "et": None,
                      "cps2": c2_ps.tile([128, N], F32, tag="c2",
                                         name=f"cps2_{hl}")}
            for qt in range(NT):
                emit_head_qt(hl, qt)
            emit_ctx2(hl, NT // 2 - 1)
            emit_epilogue_a(hl)
            for qt in range(NT - 2, NT):
                emit_transpose(hl, qt)
            if hl == 0:
                # x tiles retire with head 0's V projections; reuse their
                # SBUF for the gathered-context buffers
                xb_stack.close()
                cm_pool = p2.enter_context(tc.tile_pool(name="cm", bufs=1))
                cm["1"] = cm_pool.tile([128, CT, N], F8, tag="ctxm1",
                                       name="ctxm1")
                cm["2"] = cm_pool.tile([128, CT, N], F8, tag="ctxm2",
                                       name="ctxm2")
        # residual prefetch: the xr loads ride the rings during the last
        # head's epilogue + gather instead of competing with the drain
        xrts = {}
        for si, xr in ((0, x2r), (1, x1r)):
            for m in range(2):
                xrts[si, m] = persist.tile([128, N], F32, tag=f"xr{si}{m}",
                                           name=f"xr{si}_{m}")
                eng = nc.sync if m == 0 else nc.scalar
                eng.dma_start(xrts[si, m][:, :], xr[m, :, :])

        # epilogue of the last head: single full gather once ctx1 is done
        emit_epilogue_b(HL - 1)
        for step in range(12):
            emit_ctx1_step(HL - 1, step)
        emit_gather((HL - 1,))

        p2.close()

        # ---- P3: output projections + residual ----
        # 1024-wide half-tiles, 4 psum bufs: the early DR chains (channel
        # blocks {0,1} = heads 0-1, gathered long ago) run on the PE while
        # the final collective is in flight; late chains (blocks {2,3})
        # land right after its cm writes, and each half drains through
        # DVE+DMA independently.
        p3 = ExitStack()
        o_ps = p3.enter_context(tc.tile_pool(name="o_ps", bufs=4, space="PSUM"))
        out_pool = p3.enter_context(tc.tile_pool(name="outp", bufs=3))
        halves = []
        for si, (w_s, cmt, oo) in enumerate(((wo2_s, cm["2"], o2),
                                             (wo1_s, cm["1"], o1))):
            for m in range(2):
                for hf in range(2):
                    halves.append((si, m, hf, w_s, cmt, oo,
                                   o_ps.tile([128, 1024], F32, tag="o",
                                             name=f"ops{si}_{m}_{hf}")))

        def emit_oproj_q(si, m, hf, w_s, cmt, ps, tp):
            for c in range(2):
                ch = 2 * hf + c
                nc.tensor.matmul(
                    ps[:, c * 512:(c + 1) * 512],
                    w_s[:, 2 * tp:2 * tp + 2, m * 128:(m + 1) * 128],
                    cmt[:, 2 * tp:2 * tp + 2, ch * 512:(ch + 1) * 512],
                    start=(tp == 0), stop=(tp == 1), perf_mode=DR)

        for si, m, hf, w_s, cmt, oo, ps in halves[:4]:
            emit_oproj_q(si, m, hf, w_s, cmt, ps, 0)
        for i, (si, m, hf, w_s, cmt, oo, ps) in enumerate(halves):
            if i >= 4:
                emit_oproj_q(si, m, hf, w_s, cmt, ps, 0)
            emit_oproj_q(si, m, hf, w_s, cmt, ps, 1)
            cs = slice(hf * 1024, (hf + 1) * 1024)
            ot = out_pool.tile([128, 1024], BF16, tag="ot",
                               name=f"ot{si}_{m}_{hf}")
            nc.vector.tensor_add(ot[:, :], ps[:, :], xrts[si, m][:, cs])
            eng = nc.sync if (m + hf) % 2 == 0 else nc.scalar
            eng.dma_start(oo[m, :, cs], ot[:, :])
        p3.close()


_NC_CACHE = None


def _get_nc():
    global _NC_CACHE
    if _NC_CACHE is None:
        _NC_CACHE = _build()
    return _NC_CACHE


def _f8(a):
    return np.clip(np.asarray(a, np.float32), -240.0, 240.0).astype(_F8)


def _in_maps(x1, x2, Wq, bq, Wk, bk, Wv1, bv1, Wv2, bv2, Wo1, bo1, Wo2, bo2):
    x1f = np.asarray(x1, np.float32).reshape(B, C, N)
    x2f = np.asarray(x2, np.float32).reshape(B, C, N)
    in_maps = []
    for c in range(N_CORES):
        b, hq = c // 2, c % 2
        sl = slice(CL * hq, CL * hq + CL)

        def wslice(W, reorder=False, scale=WSC):
            a = np.asarray(W, np.float32)[:, sl].reshape(CT, 128, CL)
            if reorder:
                a = a[[0, 2, 1, 3]]
            return _f8(np.ascontiguousarray(a.transpose(1, 0, 2)) * scale)

        m = {
            "x1b": _f8(x1f[b].reshape(CT, 128, N)),
            "x2b": _f8(x2f[b].reshape(CT, 128, N)),
            "wq": wslice(Wq), "wk": wslice(Wk),
            "wv1": wslice(Wv1), "wv2": wslice(Wv2),
            "wo1": wslice(Wo1, True, OSC), "wo2": wslice(Wo2, True, OSC),
            "bq": np.ascontiguousarray(
                np.asarray(bq, np.float32)[sl].reshape(2, 128).T).reshape(128, 2, 1),
            "bk": np.ascontiguousarray(
                np.asarray(bk, np.float32)[sl].reshape(2, 128).T).reshape(128, 2, 1),
            "bv1": _f8(np.asarray(bv1, np.float32)[sl].reshape(1, CL) * WSC),
            "bv2": _f8(np.asarray(bv2, np.float32)[sl].reshape(1, CL) * WSC),
            "x1r": (x1f[b, sl, :] + np.asarray(bo1, np.float32)[sl, None]
                    ).reshape(2, 128, N),
            "x2r": (x2f[b, sl, :][:, SIGMA]
                    + np.asarray(bo2, np.float32)[sl, None]
                    ).reshape(2, 128, N),
        }
        in_maps.append(m)
    return in_maps


def _unshard(res):
    o1 = np.empty((B, C, N), np.float32)
    o2 = np.empty((B, C, N), np.float32)
    for c in range(N_CORES):
        b, hq = c // 2, c % 2
        sl = slice(CL * hq, CL * hq + CL)
        o1[b, sl, :] = np.asarray(res[c]["o1"], np.float32).reshape(CL, N)
        o2[b, sl, :][:, SIGMA] = np.asarray(res[c]["o2"],
                                            np.float32).reshape(CL, N)
    shape = (B, C, 8, 16, 16)
    return o1.reshape(shape), o2.reshape(shape)


def kernel(**inputs):
    in_maps = _in_maps(**inputs)
    nc = _get_nc()
    res = run_bass_kernel_spmd(nc, in_maps, list(range(N_CORES))).results
    return _unshard(res)


# revision 45
# speedup vs baseline: 1.1388x; 1.1388x over previous
"""Trainium2 Bass kernel for the dual-softmax cross-attention module.

Sharding: 8 cores = batch (4) x head-half (2).  Core c handles batch c//2 and
heads 4*(c%2) .. 4*(c%2)+4.  Each core computes Q/K/V projections for its
head-group, the 2048x2048 score matrix per head, one shared E = exp(s/8)
(both softmaxes are shift-invariant; scores are O(1) so no max subtraction),
contexts for both streams, exchanges context halves with its pair core via a
2-core AllGather, and produces a disjoint 256-channel slice of both outputs.

fp8 (e4m3) edition: all matmul operands are fp8 with fp32 PSUM accumulation.
Chained-contraction matmuls (QKV/O projections, ctx1, ctx2) use DoubleRow
perf mode (two 128-row K blocks per instruction).  E = exp(s/8) is written
as fp8 and transposed for ctx1 via 2-byte DMA transposes of BYTE PAIRS; a
k-side interleave permutation sigma (kslot = 256a+2p+ko <-> token
256a+128ko+p) makes the transposed byte pairs land exactly on v1's natural
token blocks, so ctx1's DoubleRow operands line up with no data shuffles.
The k-side permutation is compensated host-side on x2's residual and o2.
Weights are scaled x32 for e4m3 range; projections un-scale via ACT.
"""

import sys

for _p in ("/opt/trn_rl_repo", "/opt/pypackages"):
    if _p not in sys.path:
        sys.path.insert(0, _p)

import numpy as np
import ml_dtypes

import concourse.bass as bass
import concourse.tile as tile
from concourse import bacc, mybir
from concourse.bass_utils import run_bass_kernel_spmd

F32 = mybir.dt.float32
BF16 = mybir.dt.bfloat16
F8 = mybir.dt.float8e4
AF = mybir.ActivationFunctionType
AX = mybir.AxisListType
DR = mybir.MatmulPerfMode.DoubleRow

N_CORES = 8
B = 4          # batch
C = 512        # channels
N = 2048       # tokens (8*16*16)
H = 8          # heads
DH = 64        # head dim
HL = 4         # heads per core
CL = 256       # channels per core (head-group)
NT = N // 128  # 16 token tiles
CT = C // 128  # 4 channel tiles
WSC = 32.0     # fp8 weight pre-scale (QKV projections)
ISC = 1.0 / WSC
OSC = 4.0      # fp8 Wo pre-scale; gathered ctx is stored as ctx/OSC so the
OISC = 1.0 / OSC  # output projection psum needs no un-scaling

_F8 = ml_dtypes.float8_e4m3
_BF = ml_dtypes.bfloat16

# k-side interleave: kslot -> token
_ks = np.arange(N)
SIGMA = ((_ks >> 8) << 8) + ((_ks & 1) << 7) + ((_ks & 255) >> 1)


def _build():
    nc = bacc.Bacc("TRN2", target_bir_lowering=False, debug=False,
                   num_devices=N_CORES)

    def din(name, shape, dt=F8):
        return nc.dram_tensor(name, shape, dt, kind="ExternalInput").ap()

    x1b = din("x1b", [CT, 128, N])          # x1[b] channel-major, fp8
    x2b = din("x2b", [CT, 128, N])
    wq = din("wq", [128, CT, CL])           # column slice of Wq*32, pre-permuted
    wk = din("wk", [128, CT, CL])
    wv1 = din("wv1", [128, CT, CL])
    wv2 = din("wv2", [128, CT, CL])
    wo1 = din("wo1", [128, CT, CL])         # Wo*32 columns, CT blocks in [0,2,1,3]
    wo2 = din("wo2", [128, CT, CL])
    bq = din("bq", [128, 2, 1], F32)        # bias slices per M-tile (true scale)
    bk = din("bk", [128, 2, 1], F32)
    bv1 = din("bv1", [1, CL])               # 32*bv, fp8
    bv2 = din("bv2", [1, CL])
    x1r = din("x1r", [2, 128, N], F32)      # x1[b] residual slice + bo1
    x2r = din("x2r", [2, 128, N], F32)      # sigma-permuted + bo2

    o1 = nc.dram_tensor("o1", [2, 128, N], BF16, kind="ExternalOutput").ap()
    o2 = nc.dram_tensor("o2", [2, 128, N], BF16, kind="ExternalOutput").ap()

    with tile.TileContext(nc) as tc:
        _emit(nc, tc, locals())
    nc.compile()
    return nc


def _emit(nc, tc, t):
    x1b, x2b = t["x1b"], t["x2b"]
    wq, wk, wv1, wv2 = t["wq"], t["wk"], t["wv1"], t["wv2"]
    wo1, wo2 = t["wo1"], t["wo2"]
    bq, bk, bv1, bv2 = t["bq"], t["bk"], t["bv1"], t["bv2"]
    x1r, x2r, o1, o2 = t["x1r"], t["x2r"], t["o1"], t["o2"]

    from contextlib import ExitStack
    ctx = ExitStack()
    with ctx:
        persist = ctx.enter_context(tc.tile_pool(name="persist", bufs=1))
        small = ctx.enter_context(tc.tile_pool(name="small", bufs=8))
        vp_pool = ctx.enter_context(tc.tile_pool(name="vp", bufs=4))
        dram = ctx.enter_context(tc.tile_pool(name="dram", bufs=2, space="DRAM"))

        # ---- persistent SBUF tensors ----
        w_all = persist.tile([128, 6, CT, CL], F8, tag="wall")
        wq_s, wk_s, wv1_s, wv2_s, wo1_s, wo2_s = (w_all[:, i, :, :]
                                                  for i in range(6))
        bqk_s = persist.tile([128, 4, 1], F32, tag="bqk")
        bq_s, bk_s = bqk_s[:, 0:2, :], bqk_s[:, 2:4, :]
        ones_full = persist.tile([128, N], F8, tag="ones", name="ones_full")
        ones_s = ones_full[0:1, :]
        vb_s = persist.tile([1, 2, CL], F8, tag="vb")
        bv1_s, bv2_s = vb_s[:, 0, :], vb_s[:, 1, :]
        miscb = persist.tile([128, 8], BF16, tag="miscb")
        onec_s = miscb[:, 0:1]
        qt_s = persist.tile([128, 2, N], F8, tag="qt")    # Q^T  (chan-major)
        kt_s = persist.tile([128, 2, N], F8, tag="kt")    # K^T  (sigma k-order)
        v1tok = persist.tile([128, NT, CL], F8, tag="v1tok")  # token-major V1
        v2tok = persist.tile([128, NT, CL], F8, tag="v2tok")
        cm = {}  # gathered ctx^T tiles; pool opened once xb tiles retire

        # Q/K weights first (they gate the first projections); V/O later
        nc.sync.dma_start(w_all[:, 0, :, :], wq[:, :, :])
        nc.scalar.dma_start(w_all[:, 1, :, :], wk[:, :, :])
        nc.scalar.dma_start(bq_s[:, :, :], bq[:, :, :])
        nc.scalar.dma_start(bk_s[:, :, :], bk[:, :, :])
        nc.vector.memset(ones_s[:, :], 1.0)
        nc.vector.memset(onec_s[:, :], 1.0)

        # ---- P1: x loads + Q/K projections (V projections are interleaved
        # into head 0's qtile loop, using the then-idle ctx1 psum slot) ----
        p2 = ExitStack()
        eslab = p2.enter_context(tc.tile_pool(name="eslab", bufs=10))
        et_pool = p2.enter_context(tc.tile_pool(name="et", bufs=1))
        gsrc_pool = p2.enter_context(tc.tile_pool(name="gsrc", bufs=4))
        csrow_pool = p2.enter_context(tc.tile_pool(name="csrow", bufs=1))
        p1 = ExitStack()
        pj_ps = p1.enter_context(tc.tile_pool(name="pj_ps", bufs=2, space="PSUM"))
        xb_stack = ExitStack()
        xb_pool = xb_stack.enter_context(tc.tile_pool(name="xb", bufs=2))
        xts = {}
        for xi in range(2):
            xts[xi] = xb_pool.tile([128, CT, N], F8, tag="xb", name=f"xt{xi}")
        # 512-col chunks, first-needed first: the first QK matmuls only
        # read cols 0:512 of ti 0-1, so they start ~1us after launch
        for cc in range(4):
            for xi, xb_dram in enumerate((x1b, x2b)):
                for ti in range(CT):
                    eng = nc.sync if (cc + ti + xi) % 2 == 0 else nc.scalar
                    eng.dma_start(
                        xts[xi][:, ti, cc * 512:(cc + 1) * 512],
                        xb_dram[ti, :, cc * 512:(cc + 1) * 512])
        # V/O weights + biases land behind the x tiles on the rings
        for i, src in ((2, wv1), (3, wv2), (4, wo1), (5, wo2)):
            eng = nc.sync if i % 2 == 0 else nc.scalar
            eng.dma_start(w_all[:, i, :, :], src[:, :, :])
        nc.scalar.dma_start(bv1_s[:, :], bv1[:, :])
        nc.scalar.dma_start(bv2_s[:, :], bv2[:, :])
        # chan-major Q/K:  out[cl, n] = (1/32) sum_cin 32W[cin, cl] x[cin, n]
        for xi, w_qk, b_qk, qk_dst, perm in ((0, wq_s, bq_s, qt_s, False),
                                             (1, wk_s, bk_s, kt_s, True)):
            for m in range(2):
                for half in range(2):
                    ps = pj_ps.tile([128, 1024], F32, tag="pj")
                    for ch in range(2):
                        off = half * 1024 + ch * 512
                        for tp in range(2):
                            nc.tensor.matmul(
                                ps[:, ch * 512:(ch + 1) * 512],
                                w_qk[:, 2 * tp:2 * tp + 2, m * 128:(m + 1) * 128],
                                xts[xi][:, 2 * tp:2 * tp + 2, off:off + 512],
                                start=(tp == 0), stop=(tp == 1), perf_mode=DR)
                    dst = qk_dst[:, m, half * 1024:(half + 1) * 1024]
                    src = ps[:, :]
                    if perm:
                        # sigma interleave: token 256A+128ko+p -> col 256A+2p+ko
                        dst = dst.rearrange("c (A p ko) -> c A ko p",
                                            A=4, p=128, ko=2)
                        src = src.rearrange("c (A ko p) -> c A ko p",
                                            A=4, ko=2, p=128)
                    nc.vector.tensor_scalar(
                        dst, src, ISC, b_qk[:, m, :],
                        mybir.AluOpType.mult, mybir.AluOpType.add)
        p1.close()

        def emit_v_proj(xi, w_v, b_v, v_dst, nt, vps_pool):
            # token-major V:  out[n, cl] = (1/32)(sum_cin x 32W + 32bv)
            ps = vps_pool.tile([128, 512], F32, tag="c1", name=f"vps{xi}_{nt}")
            for tp in range(2):
                nc.tensor.matmul(
                    ps[:, 0:CL],
                    xts[xi][:, 2 * tp:2 * tp + 2, nt * 128:(nt + 1) * 128],
                    w_v[:, 2 * tp:2 * tp + 2, :],
                    start=(tp == 0), stop=False, perf_mode=DR)
            nc.tensor.matmul(ps[:, 0:CL], ones_s[:, nt * 128:(nt + 1) * 128],
                             b_v[:, :], start=False, stop=True)
            nc.vector.tensor_scalar_mul(v_dst[:, nt, :], ps[:, 0:CL], ISC)

        # ---- P2: per-head attention, software-pipelined across heads ----
        sc_ps = p2.enter_context(tc.tile_pool(name="sc_ps", bufs=2, space="PSUM"))
        c2_ps = p2.enter_context(tc.tile_pool(name="c2_ps", bufs=1, space="PSUM"))
        c1_ps = p2.enter_context(tc.tile_pool(name="c1_ps", bufs=2, space="PSUM"))

        st = {}  # per-head pipeline state

        def head_slices(hl):
            g, poff = hl // 2, 64 * (hl % 2)
            return (qt_s[poff:poff + 64, g, :], kt_s[poff:poff + 64, g, :], poff)

        def emit_scores_exp(hl, qt):
            q_l, k_l, _ = head_slices(hl)
            s = st[hl]
            if qt % 2 == 0:
                s["esp"][qt // 2] = eslab.tile([128, 2, N], F8, tag="es",
                                               name=f"es{hl}_{qt // 2}")
            es = s["esp"][qt // 2][:, qt % 2, :]
            sq = small.tile([128, 24], F32, tag="sq", bufs=4,
                            name=f"sq{hl}_{qt}")
            rs_p, rs, rr = sq[:, 0:3], sq[:, 4:5], sq[:, 5:6]
            for u in range(4):
                ps = sc_ps.tile([128, 512], F32, tag="sc", name=f"sps{u}")
                nc.tensor.matmul(ps[:, :], q_l[:, qt * 128:(qt + 1) * 128],
                                 k_l[:, u * 512:(u + 1) * 512],
                                 start=True, stop=True)
                # rowsum split: chunk 0 rides the ACT fused accumulator,
                # chunks 1-3 are reduced on DVE in one op below
                nc.scalar.activation(es[:, u * 512:(u + 1) * 512], ps[:, :],
                                     AF.Exp, scale=0.125,
                                     accum_out=(rs_p[:, 0:1]
                                                if u == 0 else None))
            nc.vector.reduce_sum(out=rs_p[:, 1:2], in_=es[:, 512:2048],
                                 axis=AX.X)
            nc.vector.tensor_add(rs[:, :], rs_p[:, 0:1], rs_p[:, 1:2])
            nc.vector.reciprocal(rr[:, :], rs[:, :])
            if qt % 2 == 0:
                s["v2pk"] = vp_pool.tile([128, 2, 80], F8, tag="v2p",
                                         bufs=2, name=f"v2pk{hl}_{qt}")
                nc.vector.memset(s["v2pk"][:, :, DH:DH + 2], 1.0)
            v2p = s["v2pk"][:, qt % 2, :]
            nc.vector.tensor_scalar_mul(
                v2p[0:128, 0:DH], v2tok[:, qt, hl * DH:(hl + 1) * DH], rr[:, :])
            if qt % 2 == 1:
                s["v2pairs"][qt // 2] = s["v2pk"]

        def emit_ctx2(hl, pj):
            # one qt-pair of ctx2 via DoubleRow (also accumulates colsum row 64)
            s = st[hl]
            esp = s["esp"][pj]
            v2pk = s["v2pairs"][pj]
            for ch in range(4):
                nc.tensor.matmul(
                    s["cps2"][0:DH + 2, ch * 512:(ch + 1) * 512],
                    v2pk[:, :, 0:DH + 2],
                    esp[:, :, ch * 512:(ch + 1) * 512],
                    start=(pj == 0), stop=(pj == NT // 2 - 1), perf_mode=DR)

        def emit_transpose(hl, qt):
            # byte-pair transpose: es fp8 [128q, 2048k] viewed as bf16
            # [128, 1024] -> et[:, qt] bf16 [128, 8, 128]; et fp8 view holds
            # E^T with (token-block 2A+ko, p) at fp8 byte (p, A, 2b+ko)
            s = st[hl]
            if s["et"] is None:
                s["et"] = et_pool.tile([128, NT, 8, 128], BF16, tag="et",
                                       name=f"et{hl}")
            nc.sync.dma_start(
                s["et"][:, qt, :, :],
                s["esp"][qt // 2][:, qt % 2, :].bitcast(BF16),
                transpose=True)

        def emit_epilogue_a(hl):
            # copy colsum row out of psum FIRST (it gates the next head's
            # psum reuse), then evac ctx2
            s = st[hl]
            csrow = csrow_pool.tile([65, N], BF16, tag="csr", name=f"csr{hl}")
            s["csrow"] = csrow
            nc.vector.tensor_copy(csrow[64:65, :], s["cps2"][64:65, :])
            gs2 = gsrc_pool.tile([64, N], F8, tag="gs", name=f"gs2_{hl}")
            s["gs2"] = gs2
            nc.vector.tensor_scalar_mul(gs2[:, :], s["cps2"][0:64, :], OISC)
            s["gs1"] = gsrc_pool.tile([64, N], F8, tag="gs",
                                      name=f"gs1_{hl}")

        def emit_epilogue_b(hl):
            # colsum row -> column via 16 K=1 matmuls with sigma-strided
            # lhsT (col nt of cs_ps = colsum of natural token block nt)
            s = st[hl]
            cs_ps = sc_ps.tile([128, 512], F32, tag="sc", name=f"cs_ps{hl}")
            csr = s["csrow"][64:65, :].rearrange("r (A p ko) -> r A ko p",
                                                 A=8, p=128, ko=2)
            for nt in range(NT):
                nc.tensor.matmul(cs_ps[:, nt:nt + 1],
                                 csr[:, nt // 2, nt % 2, :],
                                 onec_s[64:65, :], start=True, stop=True)
            cr_t = small.tile([128, NT], F32, tag="cr", bufs=2, name=f"cr{hl}")
            nc.vector.reciprocal(cr_t[:, :], cs_ps[:, 0:NT])
            v1pk = vp_pool.tile([128, NT, DH], F8, tag="v1p", bufs=2,
                                name=f"v1pk{hl}")
            s["v1pk"] = v1pk
            for nt in range(NT):
                nc.vector.tensor_scalar_mul(
                    v1pk[:, nt, :], v1tok[:, nt, hl * DH:(hl + 1) * DH],
                    cr_t[:, nt:nt + 1])

        def emit_ctx1_step(hl, step):
            # step 0..11: ch = step//3, A-pair third = step%3 (3/3/2 pairs)
            s = st[hl]
            ch, third = step // 3, step % 3
            a_lo, a_hi = ((0, 3), (3, 6), (6, 8))[third]
            if third == 0:
                s["c1"][ch] = c1_ps.tile([64, 512], F32, tag="c1",
                                         name=f"c1_{hl}_{ch}")
            et8 = s["et"][:, 4 * ch:4 * (ch + 1), :, :].bitcast(F8)
            for a in range(a_lo, a_hi):
                nc.tensor.matmul(
                    s["c1"][ch][:, :],
                    s["v1pk"][:, 2 * a:2 * a + 2, :],
                    et8[:, :, a, :].rearrange("c q (b ko) -> c ko q b",
                                              b=128, ko=2),
                    start=(a == 0), stop=(a == 7), perf_mode=DR)
            if third == 2:
                nc.vector.tensor_scalar_mul(
                    s["gs1"][:, ch * 512:(ch + 1) * 512],
                    s["c1"][ch][:, :], OISC)

        def emit_gather(hls, half=None):
            # hls: heads whose ctx ships in one collective.  half (single
            # head only): 0 = ctx2 rows, 1 = ctx1 rows.  All SBUF<->DRAM
            # legs ride the gpsimd SWDGE ring so the collective's completion
            # wait never head-of-line-blocks the HWDGE rings.
            nh = len(hls)
            nr = 128 * nh if half is None else 64
            sfx = f"{'_'.join(map(str, hls))}_{half}"
            gin = dram.tile([nr, N], F8, tag="gin", name=f"gin{sfx}")
            gout = dram.tile([2, nr, N], F8, tag="gout", bufs=4,
                             name=f"gout{sfx}")
            for i, hl in enumerate(hls):
                s = st[hl]
                if half in (None, 0):
                    nc.gpsimd.dma_start(gin[i * 128:i * 128 + 64, :]
                                        if half is None else gin[0:64, :],
                                        s["gs2"][:, :])
                if half in (None, 1):
                    ro = i * 128 + 64 if half is None else 0
                    nc.gpsimd.dma_start(gin[ro:ro + 64, :], s["gs1"][:, :])
            nc.gpsimd.collective_compute(
                "AllGather", mybir.AluOpType.bypass,
                replica_groups=[[0, 1], [2, 3], [4, 5], [6, 7]],
                ins=[gin.opt()], outs=[gout.opt()])
            for r in range(2):
                for i, hl in enumerate(hls):
                    _, _, poff = head_slices(hl)
                    # cm channel-block order [0,2,1,3] (host compensates in
                    # Wo): blocks {0,1} = heads 0-1 -> early DR pair
                    tt = 2 * (hl // 2) + r
                    if half in (None, 0):
                        ro = i * 128 if half is None else 0
                        nc.gpsimd.dma_start(cm["2"][poff:poff + 64, tt, :],
                                            gout[r, ro:ro + 64, :])
                    if half in (None, 1):
                        ro = i * 128 + 64 if half is None else 0
                        nc.gpsimd.dma_start(cm["1"][poff:poff + 64, tt, :],
                                            gout[r, ro:ro + 64, :])

        def emit_head_qt(hl, qt):
            # one qtile of head hl + interleaved epilogue work of head hl-1
            # (or, for head 0, the V projections)
            if hl == 0:
                emit_v_proj(1, wv2_s, bv2_s, v2tok, qt, c1_ps)
            emit_scores_exp(hl, qt)
            if hl == 0:
                emit_v_proj(0, wv1_s, bv1_s, v1tok, qt, c1_ps)
            else:
                if qt == 1:
                    emit_epilogue_b(hl - 1)
                elif 2 <= qt <= 13:
                    emit_ctx1_step(hl - 1, qt - 2)
                elif qt == 14 and hl >= 2:
                    # heads 0+1 ship together once head 1's ctx1 is done
                    emit_gather((0, 1) if hl == 2 else (hl - 1,))
            if qt >= 2 and qt % 2 == 0:
                emit_ctx2(hl, qt // 2 - 1)
            if qt >= 2:
                emit_transpose(hl, qt - 2)

        for hl in range(HL):
            st[hl] = {"esp": {}, "v2pairs": {}, "c1": {}, "et": None,
                      "cps2": c2_ps.tile([128, N], F32, tag="c2",
                                         name=f"cps2_{hl}")}
            for qt in range(NT):
                emit_head_qt(hl, qt)
            emit_ctx2(hl, NT // 2 - 1)
            emit_epilogue_a(hl)
            for qt in range(NT - 2, NT):
                emit_transpose(hl, qt)
            if hl == 0:
                # x tiles retire with head 0's V projections; reuse their
                # SBUF for the gathered-context buffers
                xb_stack.close()
                cm_pool = p2.enter_context(tc.tile_pool(name="cm", bufs=1))
                cm["1"] = cm_pool.tile([128, CT, N], F8, tag="ctxm1",
                                       name="ctxm1")
                cm["2"] = cm_pool.tile([128, CT, N], F8, tag="ctxm2",
                                       name="ctxm2")
        # epilogue of the last head: single full gather once ctx1 is done
        emit_epilogue_b(HL - 1)
        for step in range(12):
            emit_ctx1_step(HL - 1, step)
        emit_gather((HL - 1,))

        p2.close()

        # ---- P3: output projections + residual ----
        # Emission order lets the early DR chains (channel blocks {0,1} =
        # heads 0-1, gathered long ago) run on the PE while the final
        # collective is still in flight; the late chains (blocks {2,3})
        # land right after its cm writes.
        p3 = ExitStack()
        o_ps = p3.enter_context(tc.tile_pool(name="o_ps", bufs=2, space="PSUM"))
        xr_pool = p3.enter_context(tc.tile_pool(name="xr", bufs=2))
        out_pool = p3.enter_context(tc.tile_pool(name="outp", bufs=2))
        tiles = []
        for si, (w_s, cmt, xr, oo) in enumerate(((wo2_s, cm["2"], x2r, o2),
                                                 (wo1_s, cm["1"], x1r, o1))):
            for m in range(2):
                xr_t = xr_pool.tile([128, N], F32, tag="xr",
                                    name=f"xr{si}_{m}")
                eng = nc.sync if m == 0 else nc.scalar
                eng.dma_start(xr_t[:, :], xr[m, :, :])
                tiles.append((si, m, w_s, cmt, oo, xr_t,
                              o_ps.tile([128, N], F32, tag="o",
                                        name=f"ops{si}_{m}")))

        def emit_oproj_half(si, m, w_s, cmt, ps, tp):
            for ch in range(4):
                nc.tensor.matmul(
                    ps[:, ch * 512:(ch + 1) * 512],
                    w_s[:, 2 * tp:2 * tp + 2, m * 128:(m + 1) * 128],
                    cmt[:, 2 * tp:2 * tp + 2, ch * 512:(ch + 1) * 512],
                    start=(tp == 0), stop=(tp == 1), perf_mode=DR)

        for si, m, w_s, cmt, oo, xr_t, ps in tiles[:2]:
            emit_oproj_half(si, m, w_s, cmt, ps, 0)
        for i, (si, m, w_s, cmt, oo, xr_t, ps) in enumerate(tiles):
            if i >= 2:
                emit_oproj_half(si, m, w_s, cmt, ps, 0)
            emit_oproj_half(si, m, w_s, cmt, ps, 1)
            # drain in 1024-wide halves so DVE/DMA pipeline; psum already
            # holds ctx@Wo at true scale (OSC cancels)
            for hf in range(2):
                cs = slice(hf * 1024, (hf + 1) * 1024)
                ot = out_pool.tile([128, 1024], BF16, tag="ot", bufs=3,
                                   name=f"ot{si}_{m}_{hf}")
                nc.vector.tensor_add(ot[:, :], ps[:, cs], xr_t[:, cs])
                eng = nc.sync if (m + hf) % 2 == 0 else nc.scalar
                eng.dma_start(oo[m, :, cs], ot[:, :])
        p3.close()


_NC_CACHE = None


def _get_nc():
    global _NC_CACHE
    if _NC_CACHE is None:
        _NC_CACHE = _build()
    return _NC_CACHE


def _f8(a):
    return np.clip(np.asarray(a, np.float32), -240.0, 240.0).astype(_F8)


def _in_maps(x1, x2, Wq, bq, Wk, bk, Wv1, bv1, Wv2, bv2, Wo1, bo1, Wo2, bo2):
    x1f = np.asarray(x1, np.float32).reshape(B, C, N)
    x2f = np.asarray(x2, np.float32).reshape(B, C, N)
    in_maps = []
    for c in range(N_CORES):
        b, hq = c // 2, c % 2
        sl = slice(CL * hq, CL * hq + CL)

        def wslice(W, reorder=False, scale=WSC):
            a = np.asarray(W, np.float32)[:, sl].reshape(CT, 128, CL)
            if reorder:
                a = a[[0, 2, 1, 3]]
            return _f8(np.ascontiguousarray(a.transpose(1, 0, 2)) * scale)

        m = {
            "x1b": _f8(x1f[b].reshape(CT, 128, N)),
            "x2b": _f8(x2f[b].reshape(CT, 128, N)),
            "wq": wslice(Wq), "wk": wslice(Wk),
            "wv1": wslice(Wv1), "wv2": wslice(Wv2),
            "wo1": wslice(Wo1, True, OSC), "wo2": wslice(Wo2, True, OSC),
            "bq": np.ascontiguousarray(
                np.asarray(bq, np.float32)[sl].reshape(2, 128).T).reshape(128, 2, 1),
            "bk": np.ascontiguousarray(
                np.asarray(bk, np.float32)[sl].reshape(2, 128).T).reshape(128, 2, 1),
            "bv1": _f8(np.asarray(bv1, np.float32)[sl].reshape(1, CL) * WSC),
            "bv2": _f8(np.asarray(bv2, np.float32)[sl].reshape(1, CL) * WSC),
            "x1r": (x1f[b, sl, :] + np.asarray(bo1, np.float32)[sl, None]
                    ).reshape(2, 128, N),
            "x2r": (x2f[b, sl, :][:, SIGMA]
                    + np.asarray(bo2, np.float32)[sl, None]
                    ).reshape(2, 128, N),
        }
        in_maps.append(m)
    return in_maps


def _unshard(res):
    o1 = np.empty((B, C, N), np.float32)
    o2 = np.empty((B, C, N), np.float32)
    for c in range(N_CORES):
        b, hq = c // 2, c % 2
        sl = slice(CL * hq, CL * hq + CL)
        o1[b, sl, :] = np.asarray(res[c]["o1"], np.float32).reshape(CL, N)
        o2[b, sl, :][:, SIGMA] = np.asarray(res[c]["o2"],
                                            np.float32).reshape(CL, N)
    shape = (B, C, 8, 16, 16)
    return o1.reshape(shape), o2.reshape(shape)


def kernel(**inputs):
    in_maps = _in_maps(**inputs)
    nc = _get_nc()
    res = run_bass_kernel_spmd(nc, in_maps, list(range(N_CORES))).results
    return _unshard(res)


# revision 46
# speedup vs baseline: 1.1481x; 1.0081x over previous
"""Trainium2 Bass kernel for the dual-softmax cross-attention module.

Sharding: 8 cores = batch (4) x head-half (2).  Core c handles batch c//2 and
heads 4*(c%2) .. 4*(c%2)+4.  Each core computes Q/K/V projections for its
head-group, the 2048x2048 score matrix per head, one shared E = exp(s/8)
(both softmaxes are shift-invariant; scores are O(1) so no max subtraction),
contexts for both streams, exchanges context halves with its pair core via a
2-core AllGather, and produces a disjoint 256-channel slice of both outputs.

fp8 (e4m3) edition: all matmul operands are fp8 with fp32 PSUM accumulation.
Chained-contraction matmuls (QKV/O projections, ctx1, ctx2) use DoubleRow
perf mode (two 128-row K blocks per instruction).  E = exp(s/8) is written
as fp8 and transposed for ctx1 via 2-byte DMA transposes of BYTE PAIRS; a
k-side interleave permutation sigma (kslot = 256a+2p+ko <-> token
256a+128ko+p) makes the transposed byte pairs land exactly on v1's natural
token blocks, so ctx1's DoubleRow operands line up with no data shuffles.
The k-side permutation is compensated host-side on x2's residual and o2.
Weights are scaled x32 for e4m3 range; projections un-scale via ACT.
"""

import sys

for _p in ("/opt/trn_rl_repo", "/opt/pypackages"):
    if _p not in sys.path:
        sys.path.insert(0, _p)

import numpy as np
import ml_dtypes

import concourse.bass as bass
import concourse.tile as tile
from concourse import bacc, mybir
from concourse.bass_utils import run_bass_kernel_spmd

F32 = mybir.dt.float32
BF16 = mybir.dt.bfloat16
F8 = mybir.dt.float8e4
AF = mybir.ActivationFunctionType
AX = mybir.AxisListType
DR = mybir.MatmulPerfMode.DoubleRow

N_CORES = 8
B = 4          # batch
C = 512        # channels
N = 2048       # tokens (8*16*16)
H = 8          # heads
DH = 64        # head dim
HL = 4         # heads per core
CL = 256       # channels per core (head-group)
NT = N // 128  # 16 token tiles
CT = C // 128  # 4 channel tiles
WSC = 32.0     # fp8 weight pre-scale (QKV projections)
ISC = 1.0 / WSC
OSC = 4.0      # fp8 Wo pre-scale; gathered ctx is stored as ctx/OSC so the
OISC = 1.0 / OSC  # output projection psum needs no un-scaling

_F8 = ml_dtypes.float8_e4m3
_BF = ml_dtypes.bfloat16

# k-side interleave: kslot -> token
_ks = np.arange(N)
SIGMA = ((_ks >> 8) << 8) + ((_ks & 1) << 7) + ((_ks & 255) >> 1)


def _build():
    nc = bacc.Bacc("TRN2", target_bir_lowering=False, debug=False,
                   num_devices=N_CORES)

    def din(name, shape, dt=F8):
        return nc.dram_tensor(name, shape, dt, kind="ExternalInput").ap()

    x1b = din("x1b", [CT, 128, N])          # x1[b] channel-major, fp8
    x2b = din("x2b", [CT, 128, N])
    wq = din("wq", [128, CT, CL])           # column slice of Wq*32, pre-permuted
    wk = din("wk", [128, CT, CL])
    wv1 = din("wv1", [128, CT, CL])
    wv2 = din("wv2", [128, CT, CL])
    wo1 = din("wo1", [128, CT, CL])         # Wo*32 columns, CT blocks in [0,2,1,3]
    wo2 = din("wo2", [128, CT, CL])
    bq = din("bq", [128, 2, 1], F32)        # bias slices per M-tile (true scale)
    bk = din("bk", [128, 2, 1], F32)
    bv1 = din("bv1", [1, CL])               # 32*bv, fp8
    bv2 = din("bv2", [1, CL])
    x1r = din("x1r", [2, 128, N], F32)      # x1[b] residual slice + bo1
    x2r = din("x2r", [2, 128, N], F32)      # sigma-permuted + bo2

    o1 = nc.dram_tensor("o1", [2, 128, N], BF16, kind="ExternalOutput").ap()
    o2 = nc.dram_tensor("o2", [2, 128, N], BF16, kind="ExternalOutput").ap()

    with tile.TileContext(nc) as tc:
        _emit(nc, tc, locals())
    nc.compile()
    return nc


def _emit(nc, tc, t):
    x1b, x2b = t["x1b"], t["x2b"]
    wq, wk, wv1, wv2 = t["wq"], t["wk"], t["wv1"], t["wv2"]
    wo1, wo2 = t["wo1"], t["wo2"]
    bq, bk, bv1, bv2 = t["bq"], t["bk"], t["bv1"], t["bv2"]
    x1r, x2r, o1, o2 = t["x1r"], t["x2r"], t["o1"], t["o2"]

    from contextlib import ExitStack
    ctx = ExitStack()
    with ctx:
        persist = ctx.enter_context(tc.tile_pool(name="persist", bufs=1))
        small = ctx.enter_context(tc.tile_pool(name="small", bufs=8))
        vp_pool = ctx.enter_context(tc.tile_pool(name="vp", bufs=4))
        dram = ctx.enter_context(tc.tile_pool(name="dram", bufs=2, space="DRAM"))

        # ---- persistent SBUF tensors ----
        w_all = persist.tile([128, 6, CT, CL], F8, tag="wall")
        wq_s, wk_s, wv1_s, wv2_s, wo1_s, wo2_s = (w_all[:, i, :, :]
                                                  for i in range(6))
        bqk_s = persist.tile([128, 4, 1], F32, tag="bqk")
        bq_s, bk_s = bqk_s[:, 0:2, :], bqk_s[:, 2:4, :]
        ones_full = persist.tile([128, N], F8, tag="ones", name="ones_full")
        ones_s = ones_full[0:1, :]
        vb_s = persist.tile([1, 2, CL], F8, tag="vb")
        bv1_s, bv2_s = vb_s[:, 0, :], vb_s[:, 1, :]
        miscb = persist.tile([128, 8], BF16, tag="miscb")
        onec_s = miscb[:, 0:1]
        qt_s = persist.tile([128, 2, N], F8, tag="qt")    # Q^T  (chan-major)
        kt_s = persist.tile([128, 2, N], F8, tag="kt")    # K^T  (sigma k-order)
        v1tok = persist.tile([128, NT, CL], F8, tag="v1tok")  # token-major V1
        v2tok = persist.tile([128, NT, CL], F8, tag="v2tok")
        cm = {}  # gathered ctx^T tiles; pool opened once xb tiles retire

        # Q/K weights first (they gate the first projections); V/O later
        nc.sync.dma_start(w_all[:, 0, :, :], wq[:, :, :])
        nc.scalar.dma_start(w_all[:, 1, :, :], wk[:, :, :])
        nc.scalar.dma_start(bq_s[:, :, :], bq[:, :, :])
        nc.scalar.dma_start(bk_s[:, :, :], bk[:, :, :])
        nc.vector.memset(ones_s[:, :], 1.0)
        nc.vector.memset(onec_s[:, :], 1.0)

        # ---- P1: x loads + Q/K projections (V projections are interleaved
        # into head 0's qtile loop, using the then-idle ctx1 psum slot) ----
        p2 = ExitStack()
        eslab = p2.enter_context(tc.tile_pool(name="eslab", bufs=8))
        et_pool = p2.enter_context(tc.tile_pool(name="et", bufs=1))
        gsrc_pool = p2.enter_context(tc.tile_pool(name="gsrc", bufs=4))
        csrow_pool = p2.enter_context(tc.tile_pool(name="csrow", bufs=1))
        p1 = ExitStack()
        pj_ps = p1.enter_context(tc.tile_pool(name="pj_ps", bufs=2, space="PSUM"))
        xb_stack = ExitStack()
        xb_pool = xb_stack.enter_context(tc.tile_pool(name="xb", bufs=2))
        xts = {}
        for xi, xb_dram in enumerate((x1b, x2b)):
            xts[xi] = xb_pool.tile([128, CT, N], F8, tag="xb", name=f"xt{xi}")
            for ti in range(CT):
                eng = nc.sync if (ti + xi) % 2 == 0 else nc.scalar
                eng.dma_start(xts[xi][:, ti, :], xb_dram[ti, :, :])
        # V/O weights + biases land behind the x tiles on the rings
        for i, src in ((2, wv1), (3, wv2), (4, wo1), (5, wo2)):
            eng = nc.sync if i % 2 == 0 else nc.scalar
            eng.dma_start(w_all[:, i, :, :], src[:, :, :])
        nc.scalar.dma_start(bv1_s[:, :], bv1[:, :])
        nc.scalar.dma_start(bv2_s[:, :], bv2[:, :])
        # chan-major Q/K:  out[cl, n] = (1/32) sum_cin 32W[cin, cl] x[cin, n]
        for xi, w_qk, b_qk, qk_dst, perm in ((0, wq_s, bq_s, qt_s, False),
                                             (1, wk_s, bk_s, kt_s, True)):
            for m in range(2):
                for half in range(2):
                    ps = pj_ps.tile([128, 1024], F32, tag="pj")
                    for ch in range(2):
                        off = half * 1024 + ch * 512
                        for tp in range(2):
                            nc.tensor.matmul(
                                ps[:, ch * 512:(ch + 1) * 512],
                                w_qk[:, 2 * tp:2 * tp + 2, m * 128:(m + 1) * 128],
                                xts[xi][:, 2 * tp:2 * tp + 2, off:off + 512],
                                start=(tp == 0), stop=(tp == 1), perf_mode=DR)
                    dst = qk_dst[:, m, half * 1024:(half + 1) * 1024]
                    src = ps[:, :]
                    if perm:
                        # sigma interleave: token 256A+128ko+p -> col 256A+2p+ko
                        dst = dst.rearrange("c (A p ko) -> c A ko p",
                                            A=4, p=128, ko=2)
                        src = src.rearrange("c (A ko p) -> c A ko p",
                                            A=4, ko=2, p=128)
                    nc.vector.tensor_scalar(
                        dst, src, ISC, b_qk[:, m, :],
                        mybir.AluOpType.mult, mybir.AluOpType.add)
        p1.close()

        def emit_v_proj(xi, w_v, b_v, v_dst, nt, vps_pool):
            # token-major V:  out[n, cl] = (1/32)(sum_cin x 32W + 32bv)
            ps = vps_pool.tile([128, 512], F32, tag="c1", name=f"vps{xi}_{nt}")
            for tp in range(2):
                nc.tensor.matmul(
                    ps[:, 0:CL],
                    xts[xi][:, 2 * tp:2 * tp + 2, nt * 128:(nt + 1) * 128],
                    w_v[:, 2 * tp:2 * tp + 2, :],
                    start=(tp == 0), stop=False, perf_mode=DR)
            nc.tensor.matmul(ps[:, 0:CL], ones_s[:, nt * 128:(nt + 1) * 128],
                             b_v[:, :], start=False, stop=True)
            nc.vector.tensor_scalar_mul(v_dst[:, nt, :], ps[:, 0:CL], ISC)

        # ---- P2: per-head attention, software-pipelined across heads ----
        sc_ps = p2.enter_context(tc.tile_pool(name="sc_ps", bufs=2, space="PSUM"))
        c2_ps = p2.enter_context(tc.tile_pool(name="c2_ps", bufs=1, space="PSUM"))
        c1_ps = p2.enter_context(tc.tile_pool(name="c1_ps", bufs=2, space="PSUM"))

        st = {}  # per-head pipeline state

        def head_slices(hl):
            g, poff = hl // 2, 64 * (hl % 2)
            return (qt_s[poff:poff + 64, g, :], kt_s[poff:poff + 64, g, :], poff)

        def emit_scores_exp(hl, qt):
            q_l, k_l, _ = head_slices(hl)
            s = st[hl]
            if qt % 2 == 0:
                s["esp"][qt // 2] = eslab.tile([128, 2, N], F8, tag="es",
                                               name=f"es{hl}_{qt // 2}")
            es = s["esp"][qt // 2][:, qt % 2, :]
            sq = small.tile([128, 24], F32, tag="sq", bufs=4,
                            name=f"sq{hl}_{qt}")
            rs_p, rs, rr = sq[:, 0:3], sq[:, 4:5], sq[:, 5:6]
            for u in range(4):
                ps = sc_ps.tile([128, 512], F32, tag="sc", name=f"sps{u}")
                nc.tensor.matmul(ps[:, :], q_l[:, qt * 128:(qt + 1) * 128],
                                 k_l[:, u * 512:(u + 1) * 512],
                                 start=True, stop=True)
                # rowsum split: chunk 0 rides the ACT fused accumulator,
                # chunks 1-3 are reduced on DVE in one op below
                nc.scalar.activation(es[:, u * 512:(u + 1) * 512], ps[:, :],
                                     AF.Exp, scale=0.125,
                                     accum_out=(rs_p[:, 0:1]
                                                if u == 0 else None))
            nc.vector.reduce_sum(out=rs_p[:, 1:2], in_=es[:, 512:2048],
                                 axis=AX.X)
            nc.vector.tensor_add(rs[:, :], rs_p[:, 0:1], rs_p[:, 1:2])
            nc.vector.reciprocal(rr[:, :], rs[:, :])
            if qt % 2 == 0:
                s["v2pk"] = vp_pool.tile([128, 2, 80], F8, tag="v2p",
                                         bufs=2, name=f"v2pk{hl}_{qt}")
            v2p = s["v2pk"][:, qt % 2, :]
            nc.vector.tensor_scalar_mul(
                v2p[0:128, 0:DH], v2tok[:, qt, hl * DH:(hl + 1) * DH], rr[:, :])
            nc.vector.memset(v2p[0:128, DH:DH + 2], 1.0)
            if qt % 2 == 1:
                s["v2pairs"][qt // 2] = s["v2pk"]

        def emit_ctx2(hl, pj):
            # one qt-pair of ctx2 via DoubleRow (also accumulates colsum row 64)
            s = st[hl]
            esp = s["esp"][pj]
            v2pk = s["v2pairs"][pj]
            for ch in range(4):
                nc.tensor.matmul(
                    s["cps2"][0:DH + 2, ch * 512:(ch + 1) * 512],
                    v2pk[:, :, 0:DH + 2],
                    esp[:, :, ch * 512:(ch + 1) * 512],
                    start=(pj == 0), stop=(pj == NT // 2 - 1), perf_mode=DR)

        def emit_transpose(hl, qt):
            # byte-pair transpose: es fp8 [128q, 2048k] viewed as bf16
            # [128, 1024] -> et[:, qt] bf16 [128, 8, 128]; et fp8 view holds
            # E^T with (token-block 2A+ko, p) at fp8 byte (p, A, 2b+ko)
            s = st[hl]
            if s["et"] is None:
                s["et"] = et_pool.tile([128, NT, 8, 128], BF16, tag="et",
                                       name=f"et{hl}")
            nc.sync.dma_start(
                s["et"][:, qt, :, :],
                s["esp"][qt // 2][:, qt % 2, :].bitcast(BF16),
                transpose=True)

        def emit_epilogue_a(hl):
            # copy colsum row out of psum FIRST (it gates the next head's
            # psum reuse), then evac ctx2
            s = st[hl]
            csrow = csrow_pool.tile([65, N], BF16, tag="csr", name=f"csr{hl}")
            s["csrow"] = csrow
            nc.vector.tensor_copy(csrow[64:65, :], s["cps2"][64:65, :])
            gs2 = gsrc_pool.tile([64, N], F8, tag="gs", name=f"gs2_{hl}")
            s["gs2"] = gs2
            nc.vector.tensor_scalar_mul(gs2[:, :], s["cps2"][0:64, :], OISC)
            s["gs1"] = gsrc_pool.tile([64, N], F8, tag="gs",
                                      name=f"gs1_{hl}")

        def emit_epilogue_b(hl):
            # colsum row -> column via 16 K=1 matmuls with sigma-strided
            # lhsT (col nt of cs_ps = colsum of natural token block nt)
            s = st[hl]
            cs_ps = sc_ps.tile([128, 512], F32, tag="sc", name=f"cs_ps{hl}")
            csr = s["csrow"][64:65, :].rearrange("r (A p ko) -> r A ko p",
                                                 A=8, p=128, ko=2)
            for nt in range(NT):
                nc.tensor.matmul(cs_ps[:, nt:nt + 1],
                                 csr[:, nt // 2, nt % 2, :],
                                 onec_s[64:65, :], start=True, stop=True)
            cr_t = small.tile([128, NT], F32, tag="cr", bufs=2, name=f"cr{hl}")
            nc.vector.reciprocal(cr_t[:, :], cs_ps[:, 0:NT])
            v1pk = vp_pool.tile([128, NT, DH], F8, tag="v1p", bufs=2,
                                name=f"v1pk{hl}")
            s["v1pk"] = v1pk
            for nt in range(NT):
                nc.vector.tensor_scalar_mul(
                    v1pk[:, nt, :], v1tok[:, nt, hl * DH:(hl + 1) * DH],
                    cr_t[:, nt:nt + 1])

        def emit_ctx1_step(hl, step):
            # step 0..11: ch = step//3, A-pair third = step%3 (3/3/2 pairs)
            s = st[hl]
            ch, third = step // 3, step % 3
            a_lo, a_hi = ((0, 3), (3, 6), (6, 8))[third]
            if third == 0:
                s["c1"][ch] = c1_ps.tile([64, 512], F32, tag="c1",
                                         name=f"c1_{hl}_{ch}")
            et8 = s["et"][:, 4 * ch:4 * (ch + 1), :, :].bitcast(F8)
            for a in range(a_lo, a_hi):
                nc.tensor.matmul(
                    s["c1"][ch][:, :],
                    s["v1pk"][:, 2 * a:2 * a + 2, :],
                    et8[:, :, a, :].rearrange("c q (b ko) -> c ko q b",
                                              b=128, ko=2),
                    start=(a == 0), stop=(a == 7), perf_mode=DR)
            if third == 2:
                nc.vector.tensor_scalar_mul(
                    s["gs1"][:, ch * 512:(ch + 1) * 512],
                    s["c1"][ch][:, :], OISC)

        def emit_gather(hls, half=None):
            # hls: heads whose ctx ships in one collective.  half (single
            # head only): 0 = ctx2 rows, 1 = ctx1 rows.  All SBUF<->DRAM
            # legs ride the gpsimd SWDGE ring so the collective's completion
            # wait never head-of-line-blocks the HWDGE rings.
            nh = len(hls)
            nr = 128 * nh if half is None else 64
            sfx = f"{'_'.join(map(str, hls))}_{half}"
            gin = dram.tile([nr, N], F8, tag="gin", name=f"gin{sfx}")
            gout = dram.tile([2, nr, N], F8, tag="gout", bufs=4,
                             name=f"gout{sfx}")
            for i, hl in enumerate(hls):
                s = st[hl]
                if half in (None, 0):
                    nc.gpsimd.dma_start(gin[i * 128:i * 128 + 64, :]
                                        if half is None else gin[0:64, :],
                                        s["gs2"][:, :])
                if half in (None, 1):
                    ro = i * 128 + 64 if half is None else 0
                    nc.gpsimd.dma_start(gin[ro:ro + 64, :], s["gs1"][:, :])
            nc.gpsimd.collective_compute(
                "AllGather", mybir.AluOpType.bypass,
                replica_groups=[[0, 1], [2, 3], [4, 5], [6, 7]],
                ins=[gin.opt()], outs=[gout.opt()])
            for r in range(2):
                for i, hl in enumerate(hls):
                    _, _, poff = head_slices(hl)
                    # cm channel-block order [0,2,1,3] (host compensates in
                    # Wo): blocks {0,1} = heads 0-1 -> early DR pair
                    tt = 2 * (hl // 2) + r
                    if half in (None, 0):
                        ro = i * 128 if half is None else 0
                        nc.gpsimd.dma_start(cm["2"][poff:poff + 64, tt, :],
                                            gout[r, ro:ro + 64, :])
                    if half in (None, 1):
                        ro = i * 128 + 64 if half is None else 0
                        nc.gpsimd.dma_start(cm["1"][poff:poff + 64, tt, :],
                                            gout[r, ro:ro + 64, :])

        def emit_head_qt(hl, qt):
            # one qtile of head hl + interleaved epilogue work of head hl-1
            # (or, for head 0, the V projections)
            if hl == 0:
                emit_v_proj(1, wv2_s, bv2_s, v2tok, qt, c1_ps)
            emit_scores_exp(hl, qt)
            if hl == 0:
                emit_v_proj(0, wv1_s, bv1_s, v1tok, qt, c1_ps)
            else:
                if qt == 1:
                    emit_epilogue_b(hl - 1)
                elif 2 <= qt <= 13:
                    emit_ctx1_step(hl - 1, qt - 2)
                elif qt == 14 and hl >= 2:
                    # heads 0+1 ship together once head 1's ctx1 is done
                    emit_gather((0, 1) if hl == 2 else (hl - 1,))
            if qt >= 2 and qt % 2 == 0:
                emit_ctx2(hl, qt // 2 - 1)
            if qt >= 2:
                emit_transpose(hl, qt - 2)

        for hl in range(HL):
            st[hl] = {"esp": {}, "v2pairs": {}, "c1": {}, "et": None,
                      "cps2": c2_ps.tile([128, N], F32, tag="c2",
                                         name=f"cps2_{hl}")}
            for qt in range(NT):
                emit_head_qt(hl, qt)
            emit_ctx2(hl, NT // 2 - 1)
            emit_epilogue_a(hl)
            for qt in range(NT - 2, NT):
                emit_transpose(hl, qt)
            if hl == 0:
                # x tiles retire with head 0's V projections; reuse their
                # SBUF for the gathered-context buffers
                xb_stack.close()
                cm_pool = p2.enter_context(tc.tile_pool(name="cm", bufs=1))
                cm["1"] = cm_pool.tile([128, CT, N], F8, tag="ctxm1",
                                       name="ctxm1")
                cm["2"] = cm_pool.tile([128, CT, N], F8, tag="ctxm2",
                                       name="ctxm2")
        # epilogue of the last head: single full gather once ctx1 is done
        emit_epilogue_b(HL - 1)
        for step in range(12):
            emit_ctx1_step(HL - 1, step)
        emit_gather((HL - 1,))

        p2.close()

        # ---- P3: output projections + residual ----
        # Emission order lets the early DR chains (channel blocks {0,1} =
        # heads 0-1, gathered long ago) run on the PE while the final
        # collective is still in flight; the late chains (blocks {2,3})
        # land right after its cm writes.
        p3 = ExitStack()
        o_ps = p3.enter_context(tc.tile_pool(name="o_ps", bufs=2, space="PSUM"))
        xr_pool = p3.enter_context(tc.tile_pool(name="xr", bufs=2))
        out_pool = p3.enter_context(tc.tile_pool(name="outp", bufs=2))
        tiles = []
        for si, (w_s, cmt, xr, oo) in enumerate(((wo2_s, cm["2"], x2r, o2),
                                                 (wo1_s, cm["1"], x1r, o1))):
            for m in range(2):
                xr_t = xr_pool.tile([128, N], F32, tag="xr",
                                    name=f"xr{si}_{m}")
                eng = nc.sync if m == 0 else nc.scalar
                eng.dma_start(xr_t[:, :], xr[m, :, :])
                tiles.append((si, m, w_s, cmt, oo, xr_t,
                              o_ps.tile([128, N], F32, tag="o",
                                        name=f"ops{si}_{m}")))

        def emit_oproj_half(si, m, w_s, cmt, ps, tp):
            for ch in range(4):
                nc.tensor.matmul(
                    ps[:, ch * 512:(ch + 1) * 512],
                    w_s[:, 2 * tp:2 * tp + 2, m * 128:(m + 1) * 128],
                    cmt[:, 2 * tp:2 * tp + 2, ch * 512:(ch + 1) * 512],
                    start=(tp == 0), stop=(tp == 1), perf_mode=DR)

        for si, m, w_s, cmt, oo, xr_t, ps in tiles[:2]:
            emit_oproj_half(si, m, w_s, cmt, ps, 0)
        for i, (si, m, w_s, cmt, oo, xr_t, ps) in enumerate(tiles):
            if i >= 2:
                emit_oproj_half(si, m, w_s, cmt, ps, 0)
            emit_oproj_half(si, m, w_s, cmt, ps, 1)
            # drain in 1024-wide halves so DVE/DMA pipeline; psum already
            # holds ctx@Wo at true scale (OSC cancels)
            for hf in range(2):
                cs = slice(hf * 1024, (hf + 1) * 1024)
                ot = out_pool.tile([128, 1024], BF16, tag="ot", bufs=3,
                                   name=f"ot{si}_{m}_{hf}")
                nc.vector.tensor_add(ot[:, :], ps[:, cs], xr_t[:, cs])
                eng = nc.sync if (m + hf) % 2 == 0 else nc.scalar
                eng.dma_start(oo[m, :, cs], ot[:, :])
        p3.close()


_NC_CACHE = None


def _get_nc():
    global _NC_CACHE
    if _NC_CACHE is None:
        _NC_CACHE = _build()
    return _NC_CACHE


def _f8(a):
    return np.clip(np.asarray(a, np.float32), -240.0, 240.0).astype(_F8)


def _in_maps(x1, x2, Wq, bq, Wk, bk, Wv1, bv1, Wv2, bv2, Wo1, bo1, Wo2, bo2):
    x1f = np.asarray(x1, np.float32).reshape(B, C, N)
    x2f = np.asarray(x2, np.float32).reshape(B, C, N)
    in_maps = []
    for c in range(N_CORES):
        b, hq = c // 2, c % 2
        sl = slice(CL * hq, CL * hq + CL)

        def wslice(W, reorder=False, scale=WSC):
            a = np.asarray(W, np.float32)[:, sl].reshape(CT, 128, CL)
            if reorder:
                a = a[[0, 2, 1, 3]]
            return _f8(np.ascontiguousarray(a.transpose(1, 0, 2)) * scale)

        m = {
            "x1b": _f8(x1f[b].reshape(CT, 128, N)),
            "x2b": _f8(x2f[b].reshape(CT, 128, N)),
            "wq": wslice(Wq), "wk": wslice(Wk),
            "wv1": wslice(Wv1), "wv2": wslice(Wv2),
            "wo1": wslice(Wo1, True, OSC), "wo2": wslice(Wo2, True, OSC),
            "bq": np.ascontiguousarray(
                np.asarray(bq, np.float32)[sl].reshape(2, 128).T).reshape(128, 2, 1),
            "bk": np.ascontiguousarray(
                np.asarray(bk, np.float32)[sl].reshape(2, 128).T).reshape(128, 2, 1),
            "bv1": _f8(np.asarray(bv1, np.float32)[sl].reshape(1, CL) * WSC),
            "bv2": _f8(np.asarray(bv2, np.float32)[sl].reshape(1, CL) * WSC),
            "x1r": (x1f[b, sl, :] + np.asarray(bo1, np.float32)[sl, None]
                    ).reshape(2, 128, N),
            "x2r": (x2f[b, sl, :][:, SIGMA]
                    + np.asarray(bo2, np.float32)[sl, None]
                    ).reshape(2, 128, N),
        }
        in_maps.append(m)
    return in_maps


def _unshard(res):
    o1 = np.empty((B, C, N), np.float32)
    o2 = np.empty((B, C, N), np.float32)
    for c in range(N_CORES):
        b, hq = c // 2, c % 2
        sl = slice(CL * hq, CL * hq + CL)
        o1[b, sl, :] = np.asarray(res[c]["o1"], np.float32).reshape(CL, N)
        o2[b, sl, :][:, SIGMA] = np.asarray(res[c]["o2"],
                                            np.float32).reshape(CL, N)
    shape = (B, C, 8, 16, 16)
    return o1.reshape(shape), o2.reshape(shape)


def kernel(**inputs):
    in_maps = _in_maps(**inputs)
    nc = _get_nc()
    res = run_bass_kernel_spmd(nc, in_maps, list(range(N_CORES))).results
    return _unshard(res)
